# revision 1
# baseline (speedup 1.0000x reference)
"""Trainium2 Bass kernel for a 2-layer masked LSTM + FC + sigmoid head.

Problem shapes (hardcoded): B=1024, T=512, I=16, H=64.
Sharding: pure data parallel, batch 1024 -> 8 cores x 128.

Per-core design notes
---------------------
Gate pre-activations for both layers live in one PSUM tile [128, 256]
(gate rows on partitions, layer-blocks on free: cols 0:128 = layer-0
batch, 128:256 = layer-1 batch). Layer 1 runs TWO timesteps behind
layer 0 (at super-step k, L0 processes t=k, L1 processes t=k-2), so
every layer-1 matmul reads state that is >= 2 iterations old and drops
off the serial critical chain; only L0's two hh-GEMMs + the L0 slice of
the elementwise chain remain serial.

Hidden state is partition-packed: hb [128, 128] with h0 in rows 0:64 and
h1 in rows 64:128, so layer 1's two GEMMs (W_ih1 @ h0 + W_hh1 @ h1)
merge into ONE K=128 matmul. All matmul paths use the fp32r dtype (1 PE
pass, 2x faster than fp32 which lowers to 2 passes; ~1e-5 rel err).

Biases: layer-0 bias rides a constant-one row (row 16) of the staged x
block inside the K=32 x-projection matmul; layer-1 bias is a K=32 matmul
against an all-ones rhs (K=32 measured ~3x faster than K=1/K=2 shapes).

Masking: the reference freezes (h, c) where mask==0. Because the mask is
a length-prefix mask, the unmasked recurrence agrees with the masked one
for t < len(b), and the final layer-1 hidden equals h1 at t = len(b)-1.
So we run unmasked and accumulate h2_final = sum_t d_t * h1(t) with the
one-hot indicator d[b, t] = mask[b, t] - mask[b, t+1] (host-computed).
d_t is broadcast across partitions with gpsimd.partition_broadcast and
the multiply-accumulate runs on the otherwise-idle GPSIMD engine.

x is pre-transposed on host into xs[p, c] = x[b, t, i] at
p = (t%4)*32 + i, c = (t//4)*128 + b (row 16 of each 32-block is 1.0 for
the bias rider; rows 17:32 zero). Each step GPSIMD stages the [32, 128]
x-block to a base-0 tile so every matmul in a PSUM accumulation group
uses PE row-group 0 (mixing row-groups inside one group wedges the
device). PSUM groups in one tile are opened and closed sequentially
(L1-cols group fully closes before the L0-cols group opens).
"""

from contextlib import ExitStack

import numpy as np

import concourse.bass as bass
import concourse.tile as tile
from concourse import bacc, mybir
from concourse import bass_utils

F32 = mybir.dt.float32
F32R = mybir.dt.float32r
AF = mybir.ActivationFunctionType
OP = mybir.AluOpType

B, T, I, H = 1024, 512, 16, 64
NCORES = 8
BL = B // NCORES  # 128 batch per core

_BUILT = {}


def _build_program(t_steps: int):
    """Build the Bass program (single-core SPMD body). Returns compiled nc."""
    nc = bacc.Bacc(
        "TRN2",
        target_bir_lowering=False,
        debug=False,
        enable_asserts=False,
        num_devices=NCORES,
    )

    # ---- DRAM I/O ----
    d_xs = nc.dram_tensor("xs", [128, (t_steps // 4) * 128], F32R, kind="ExternalInput")
    d_ds = nc.dram_tensor("ds", [128, (t_steps // 4) * 128], F32R, kind="ExternalInput")
    d_w = {}
    for name, k in [
        ("wxif0", 32), ("wxog0", 32),
        ("whif0", 64), ("whog0", 64),
        ("wbif1", 128), ("wbog1", 128),
        ("b1if", 32), ("b1og", 32),
        ("onesb", 32),
    ]:
        d_w[name] = nc.dram_tensor(name, [k, 128], F32R, kind="ExternalInput")
    d_fct = nc.dram_tensor("fct", [64, 1], F32R, kind="ExternalInput")
    d_ones4 = nc.dram_tensor("ones4", [128, 64], F32R, kind="ExternalInput")
    d_zini = nc.dram_tensor("zini", [64, 128], F32R, kind="ExternalInput")
    d_fcb = nc.dram_tensor("fcb", [1, 1], F32, kind="ExternalInput")
    d_out = nc.dram_tensor("out", [1, 128], F32, kind="ExternalOutput")

    with tile.TileContext(nc) as tc, ExitStack() as ctx:
        pconst = ctx.enter_context(tc.tile_pool(name="const", bufs=1))
        pstate = ctx.enter_context(tc.tile_pool(name="state", bufs=3))
        ppsum = ctx.enter_context(tc.tile_pool(name="psum", bufs=2, space="PSUM"))
        pwork = ctx.enter_context(tc.tile_pool(name="work", bufs=3))

        # ---- persistent SBUF: inputs ----
        xs = pconst.tile([128, (t_steps // 4) * 128], F32R, tag="xs")
        n_xchunks = 8
        xw = (t_steps // 4) * 128 // n_xchunks
        for j in range(n_xchunks):
            nc.sync.dma_start(xs[:, j * xw:(j + 1) * xw], d_xs.ap()[:, j * xw:(j + 1) * xw])
        ds = pconst.tile([128, (t_steps // 4) * 128], F32R, tag="ds")
        for j in range(4):
            dw = (t_steps // 4) * 128 // 4
            nc.sync.dma_start(ds[:, j * dw:(j + 1) * dw], d_ds.ap()[:, j * dw:(j + 1) * dw])

        w = {}
        for name, k in [
            ("wxif0", 32), ("wxog0", 32),
            ("whif0", 64), ("whog0", 64),
            ("wbif1", 128), ("wbog1", 128),
            ("b1if", 32), ("b1og", 32),
            ("onesb", 32),
        ]:
            w[name] = pconst.tile([k, 128], F32R, tag=name, name=name)
            nc.sync.dma_start(w[name][:], d_w[name].ap()[:])
        ones4 = pconst.tile([128, 64], F32R, tag="ones4")
        nc.sync.dma_start(ones4[:], d_ones4.ap()[:])
        fct = pconst.tile([64, 1], F32R, tag="fct")
        nc.sync.dma_start(fct[:], d_fct.ap()[:])
        fcb = pconst.tile([1, 1], F32, tag="fcb")
        nc.sync.dma_start(fcb[:], d_fcb.ap()[:])

        # ---- state ----
        # hb ring: [128, 128] rows 0:64 = h0(t), rows 64:128 = h1(t-1);
        # iteration k reads hb[k%4] for L1 (state 2-3 iters old) and
        # hb[(k+1)%4] rows 0:64 for L0's hh (1 iter old). Writers:
        # hmult0(k) -> hb[(k+2)%4][0:64]; hmult1(k) -> hb[(k+1)%4][64:128].
        hb = []
        for i in range(4):
            t_ = pstate.tile([128, 128], F32R, tag=f"hb{i}", name=f"hb{i}")
            nc.sync.dma_start(t_[0:64, :], d_zini.ap()[:])
            nc.sync.dma_start(t_[64:128, :], d_zini.ap()[:])
            hb.append(t_)
        # gc per layer: [128, 128] rows 0:64 = tanh'd g gate (this step),
        # rows 64:128 = cell state (carried from previous step).
        gc0_cur = pstate.tile([128, 128], F32, tag="gc0")
        nc.vector.memset(gc0_cur[:], 0.0)
        gc1_cur = pstate.tile([128, 128], F32, tag="gc1")
        nc.vector.memset(gc1_cur[:], 0.0)
        h2acc = pconst.tile([64, 128], F32R, tag="h2acc")
        nc.sync.dma_start(h2acc[:], d_zini.ap()[:])

        mm = nc.tensor.matmul
        for k in range(t_steps + 2):
            l0 = k < t_steps
            l1 = 2 <= k <= t_steps + 1

            # ---------- gate GEMMs ----------
            p_if = ppsum.tile([128, 256], F32, tag="pif")
            p_og = ppsum.tile([128, 256], F32, tag="pog")

            if l1:
                # layer-1 t=k-2: all inputs >= 2 iterations old (off-chain)
                mm(p_if[:, 128:256], w["b1if"][:], w["onesb"][:], start=True, stop=False)
                mm(p_if[:, 128:256], w["wbif1"][:], hb[k % 4][:], start=False, stop=True)
                mm(p_og[:, 128:256], w["b1og"][:], w["onesb"][:], start=True, stop=False)
                mm(p_og[:, 128:256], w["wbog1"][:], hb[k % 4][:], start=False, stop=True)
            if l0:
                ph = (k % 4) * 32
                xsl = xs[ph:ph + 32, (k // 4) * 128:(k // 4) * 128 + 128]
                xst = pwork.tile([32, 128], F32R, tag="xst")
                nc.gpsimd.tensor_copy(xst[:], xsl)
                mm(p_if[:, 0:128], w["wxif0"][:], xst[:], start=True, stop=False)
                mm(p_og[:, 0:128], w["wxog0"][:], xst[:], start=True, stop=False)
                h0prev = hb[(k + 1) % 4][0:64, :]
                mm(p_og[:, 0:128], w["whog0"][:], h0prev, start=False, stop=True)
                mm(p_if[:, 0:128], w["whif0"][:], h0prev, start=False, stop=True)

            # ---------- activations (packed across layers) ----------
            ca = 0 if l0 else 128
            cb = 256 if l1 else 128
            if l0:
                gc0_next = pstate.tile([128, 128], F32, tag="gc0")
                nc.scalar.activation(gc0_next[0:64, :], p_og[64:128, 0:128], AF.Tanh)
            if l1:
                gc1_next = pstate.tile([128, 128], F32, tag="gc1")
                nc.scalar.activation(gc1_next[0:64, :], p_og[64:128, 128:256], AF.Tanh)
            g_if = pwork.tile([128, 256], F32, tag="gif")
            nc.scalar.activation(g_if[:, ca:cb], p_if[:, ca:cb], AF.Sigmoid)
            g_o = pwork.tile([64, 256], F32, tag="go")
            nc.scalar.activation(g_o[:, ca:cb], p_og[0:64, ca:cb], AF.Sigmoid)

            # ---------- cell/hidden update, per layer ----------
            # igfc cols 0:128 = i*g, cols 128:256 = f*c_prev (free-packed so
            # the add sees both SBUF inputs at base partition 0)
            hb_w0 = hb[(k + 2) % 4]
            hb_w1 = hb[(k + 1) % 4]
            if l0:
                igfc0 = pwork.tile([64, 256], F32, tag="igfc0")
                nc.vector.tensor_tensor(igfc0[:, 0:128], g_if[0:64, 0:128],
                                        gc0_next[0:64, :], OP.mult)
                nc.vector.tensor_tensor(igfc0[:, 128:256], g_if[64:128, 0:128],
                                        gc0_cur[64:128, :], OP.mult)
                nc.vector.tensor_tensor(gc0_next[64:128, :], igfc0[:, 0:128],
                                        igfc0[:, 128:256], OP.add)
                tc0 = pwork.tile([64, 128], F32, tag="tc0")
                nc.scalar.activation(tc0[:], gc0_next[64:128, :], AF.Tanh)
                nc.vector.tensor_tensor(hb_w0[0:64, :], g_o[:, 0:128], tc0[:], OP.mult)
            if l1:
                igfc1 = pwork.tile([64, 256], F32, tag="igfc1")
                nc.vector.tensor_tensor(igfc1[:, 0:128], g_if[0:64, 128:256],
                                        gc1_next[0:64, :], OP.mult)
                nc.vector.tensor_tensor(igfc1[:, 128:256], g_if[64:128, 128:256],
                                        gc1_cur[64:128, :], OP.mult)
                nc.vector.tensor_tensor(gc1_next[64:128, :], igfc1[:, 0:128],
                                        igfc1[:, 128:256], OP.add)
                tc1 = pwork.tile([64, 128], F32, tag="tc1")
                nc.scalar.activation(tc1[:], gc1_next[64:128, :], AF.Tanh)
                nc.vector.tensor_tensor(hb_w1[64:128, :], g_o[:, 128:256], tc1[:], OP.mult)

                # ---- final-step capture: h2acc += bcast(d_{k-2}) * h1 ----
                tcap = k - 2
                pc = (tcap % 4) * 32
                dsl = ds[pc:pc + 1, (tcap // 4) * 128:(tcap // 4) * 128 + 128]
                psd = ppsum.tile([64, 128], F32, tag="psd")
                mm(psd[:], ones4[pc:pc + 1, :], dsl, start=True, stop=True,
                   tile_position=(pc, 0))
                cap = pwork.tile([64, 128], F32R, tag="cap")
                nc.vector.tensor_tensor(cap[:], psd[:], hb_w1[64:128, :], OP.mult)
                nc.vector.tensor_tensor(h2acc[:], h2acc[:], cap[:], OP.add)

            if l0:
                gc0_cur = gc0_next
            if l1:
                gc1_cur = gc1_next

        # ---------- FC + sigmoid head ----------
        pfc = ppsum.tile([1, 128], F32, tag="pif")
        mm(pfc[:], fct[:], h2acc[:], start=True, stop=True)
        osb = pwork.tile([1, 128], F32, tag="osb")
        nc.scalar.activation(osb[:], pfc[:], AF.Sigmoid, bias=fcb[:, 0:1])
        nc.sync.dma_start(d_out.ap()[:], osb[:])

    nc.compile()
    return nc


def _get_program(t_steps: int):
    if t_steps not in _BUILT:
        _BUILT[t_steps] = _build_program(t_steps)
    return _BUILT[t_steps]


def _prep_core_inputs(x, dmask, weights, t_steps):
    """Host-side layout prep for one core's shard. x: [BL, T, I], dmask: [BL, T]."""
    tq = t_steps // 4
    # xs[p, c] = x[b, t, i] at p=(t%4)*32+i, c=(t//4)*128+b; row 16 = 1.0
    xpad = np.zeros((BL, t_steps, 32), np.float32)
    xpad[:, :, :I] = x
    xpad[:, :, 16] = 1.0  # bias rider row
    xs = (
        xpad.transpose(1, 2, 0)           # [t, i32, b]
        .reshape(tq, 4, 32, BL)
        .transpose(1, 2, 0, 3)            # [t%4, i32, t//4, b]
        .reshape(128, tq * 128)
    )
    xs = np.ascontiguousarray(xs)
    # ds[p, c] = d[b, t] at p=(t%4)*32, c=(t//4)*128+b
    dsb = np.zeros((128, tq * 128), np.float32)
    dv = (
        dmask.transpose(1, 0)             # [t, b]
        .reshape(tq, 4, BL)
        .transpose(1, 0, 2)               # [t%4, t//4, b]
        .reshape(4, tq * 128)
    )
    dsb[0::32][:4] = dv
    return dict(xs=xs, ds=dsb, **weights)


def _host_weights(w_ih0, w_hh0, b_ih0, b_hh0,
                  w_ih1, w_hh1, b_ih1, b_hh1, fc_w, fc_b):
    def lt(a):  # lhsT helper
        return np.ascontiguousarray(np.asarray(a, np.float32).T)

    b0 = np.asarray(b_ih0, np.float32) + np.asarray(b_hh0, np.float32)
    b1 = np.asarray(b_ih1, np.float32) + np.asarray(b_hh1, np.float32)

    def og(a):  # reorder [4H, K] gate rows -> [o; g] stacked
        return np.concatenate([a[3 * H:4 * H], a[2 * H:3 * H]], axis=0)

    def xw0(wslice, bslice):  # [128, 16] -> [32, 128] with bias rider row 16
        out = np.zeros((32, 128), np.float32)
        out[:16] = lt(wslice)
        out[16] = bslice
        return out

    def brow(bslice):  # [128] -> [32, 128] with bias in row 0
        out = np.zeros((32, 128), np.float32)
        out[0] = bslice
        return out

    wih0, whh0 = np.asarray(w_ih0), np.asarray(w_hh0)
    wih1, whh1 = np.asarray(w_ih1), np.asarray(w_hh1)
    b0og = np.concatenate([b0[3 * H:4 * H], b0[2 * H:3 * H]])
    b1og = np.concatenate([b1[3 * H:4 * H], b1[2 * H:3 * H]])
    weights = dict(
        wxif0=xw0(wih0[0:2 * H], b0[0:2 * H]),
        wxog0=xw0(og(wih0), b0og),
        whif0=lt(whh0[0:2 * H]),
        whog0=lt(og(whh0)),
        wbif1=np.concatenate([lt(wih1[0:2 * H]), lt(whh1[0:2 * H])]),
        wbog1=np.concatenate([lt(og(wih1)), lt(og(whh1))]),
        b1if=brow(b1[0:2 * H]),
        b1og=brow(b1og),
        onesb=np.ones((32, 128), np.float32),
        fct=np.ascontiguousarray(np.asarray(fc_w, np.float32).reshape(1, H).T),
        fcb=np.asarray(fc_b, np.float32).reshape(1, 1),
        zini=np.zeros((64, 128), np.float32),
        ones4=np.ones((128, 64), np.float32),
    )
    return weights


def _run(x, mask, w_ih0, w_hh0, b_ih0, b_hh0,
         w_ih1, w_hh1, b_ih1, b_hh1, fc_w, fc_b, trace=False):
    t_steps = x.shape[1]
    x = np.asarray(x, np.float32)
    mask = np.asarray(mask)

    # d[b, t] = mask[b, t] - mask[b, t+1]  (one-hot at t = len_b - 1)
    m = mask.astype(np.float32)
    d = m - np.concatenate([m[:, 1:], np.zeros((m.shape[0], 1), np.float32)], axis=1)

    weights = _host_weights(w_ih0, w_hh0, b_ih0, b_hh0,
                            w_ih1, w_hh1, b_ih1, b_hh1, fc_w, fc_b)

    nc = _get_program(t_steps)
    in_maps = []
    for c in range(NCORES):
        sl = slice(c * BL, (c + 1) * BL)
        in_maps.append(_prep_core_inputs(x[sl], d[sl], weights, t_steps))

    res = bass_utils.run_bass_kernel_spmd(nc, in_maps, core_ids=list(range(NCORES)),
                                          trace=trace)
    out = np.concatenate([res.results[c]["out"].reshape(BL) for c in range(NCORES)])
    return out.astype(np.float32), res


def kernel(**inputs):
    return _run(**inputs)[0]


def kernel_traced(**inputs):
    return _run(**inputs, trace=True)



# revision 7
# speedup vs baseline: 1.0850x; 1.0850x over previous
"""Trainium2 Bass kernel for a 2-layer masked LSTM + FC + sigmoid head.

Problem shapes (hardcoded): B=1024, T=512, I=16, H=64.
Sharding: pure data parallel, batch 1024 -> 8 cores x 128.

Per-core design (v2 — batched projections, minimal critical chain)
------------------------------------------------------------------
Gate banks: one PSUM bank [128, 512] holds a PAIR of steps per layer:
cols [256s : 256s+128] = IF gates (i rows 0:64, f rows 64:128) and
cols [256s+128 : 256s+256] = OG gates (2g rows 0:64, o rows 64:128)
for local step s in {0,1}. Two rotating banks per layer (4 banks).

Off-chain big matmuls (moving dim N=256, so fp32r runs at 1 cyc/row
instead of the 4-cyc penalty N<256 pays): per step-pair, the x
projection (L0) / h0 projection (L1) writes the strided IF (resp. OG)
columns of the bank with start=True. Biases ride a trailing ones row of
the rhs (x row 16; h0-history row 64), so activations need no bias and
the per-step recurrent matmuls add no bias twice.

Per-step recurrent matmuls: out = bank[:, 256s : 256s+256] (N=256),
rhs = [h(t-1) | zeros] for IF and [zeros | h(t-1)] for OG — the zero
half accumulates +0 into the other gate block's columns. h state lives
in a 3-tile history ring [65, 2304] with 128 zero columns interleaved
between h blocks so both rhs patterns find zeros adjacent; row 64 is a
constant ones row (bias rider for the L1 projection). h0 occupies cols
0:1152, h1 cols 1152:2304; layer 1 lags layer 0 by 2 steps.

Activations: tanh(g) is computed as 2*sigmoid(2g) - 1 with the 2x
folded into the g-gate weights/biases, so ONE sigmoid per gate-block
covers i,f,2g,o. The cell update uses scalar_tensor_tensor fusions:
ig2 = (sig - 0.5) * i  (= i*tanh(g)/2), c' = 2*ig2 + f*c.

Masked final-state capture: run unmasked; h2 += d_t * h1(t) with
one-hot d[b,t] = mask[b,t] - mask[b,t+1], host-prebroadcast to
dbc[64, T*128] and DMA-streamed; the multiply-accumulate runs as two
[64, 512] DVE ops per 4-step block.

x is DMA-streamed from DRAM as [17, 2048] chunks (16 steps), triple
buffered. All matmul paths use fp32r.
"""

from contextlib import ExitStack

import numpy as np

import concourse.bass as bass
import concourse.tile as tile
from concourse import bacc, mybir
from concourse import bass_utils

F32 = mybir.dt.float32
F32R = mybir.dt.float32r
AF = mybir.ActivationFunctionType
OP = mybir.AluOpType

B, T, I, H = 1024, 512, 16, 64
NCORES = 8
BL = B // NCORES  # 128 batch per core
LAG = 2           # layer-1 step lag
HB = 2304         # hist tile free size: 2 layers x (128 + 4*(256)) = 2*1152

_BUILT = {}


def _h0col(s):
    return 128 + 256 * (s % 4)


def _h1col(s):
    return 1152 + 128 + 256 * (s % 4)


def _blk(t):
    return (t // 4) % 3


def _build_program(t_steps: int):
    nc = bacc.Bacc(
        "TRN2",
        target_bir_lowering=False,
        debug=False,
        enable_asserts=False,
        num_devices=NCORES,
    )

    TB = t_steps * BL  # 65536
    d_xs = nc.dram_tensor("xs", [17, TB], F32R, kind="ExternalInput")
    d_dbc = nc.dram_tensor("dbc", [64, TB], F32, kind="ExternalInput")
    wnames = [
        ("wxif0", 17), ("wxog0", 17),
        ("whif0", 64), ("whog0", 64),
        ("w0if1", 65), ("w0og1", 65),
        ("whif1", 64), ("whog1", 64),
    ]
    d_w = {}
    for name, k in wnames:
        d_w[name] = nc.dram_tensor(name, [k, 128], F32R, kind="ExternalInput")
    d_fct = nc.dram_tensor("fct", [64, 1], F32R, kind="ExternalInput")
    d_fcb = nc.dram_tensor("fcb", [1, 1], F32, kind="ExternalInput")
    d_out = nc.dram_tensor("out", [1, 128], F32, kind="ExternalOutput")

    NCH = t_steps * BL // 2048  # 32 chunks of 16 steps

    with tile.TileContext(nc) as tc, ExitStack() as ctx:
        pconst = ctx.enter_context(tc.tile_pool(name="const", bufs=1))
        pstate = ctx.enter_context(tc.tile_pool(name="state", bufs=1))
        ppsum = ctx.enter_context(tc.tile_pool(name="psum", bufs=1, space="PSUM"))
        pwork = ctx.enter_context(tc.tile_pool(name="work", bufs=3))

        # ---- weights ----
        w = {}
        for name, k in wnames:
            w[name] = pconst.tile([k, 128], F32R, tag=name, name=name)
            nc.sync.dma_start(w[name][:], d_w[name].ap()[:])
        fct = pconst.tile([64, 1], F32R, tag="fct")
        nc.sync.dma_start(fct[:], d_fct.ap()[:])
        fcb = pconst.tile([1, 1], F32, tag="fcb")
        nc.sync.dma_start(fcb[:], d_fcb.ap()[:])

        # ---- streamed inputs (triple-buffered chunks of 16 steps) ----
        xsb = [pconst.tile([17, 2048], F32R, tag=f"xsb{i}", name=f"xsb{i}") for i in range(3)]
        dcb = [pconst.tile([64, 2048], F32, tag=f"dcb{i}", name=f"dcb{i}") for i in range(3)]
        for j in range(2):
            nc.sync.dma_start(xsb[j][:], d_xs.ap()[:, j * 2048:(j + 1) * 2048])
            nc.sync.dma_start(dcb[j][:], d_dbc.ap()[:, j * 2048:(j + 1) * 2048])

        # ---- state ----
        hist = []
        for i in range(3):
            t_ = pstate.tile([65, HB], F32R, tag=f"hist{i}", name=f"hist{i}")
            nc.vector.memset(t_[:].bitcast(F32), 0.0)
            nc.vector.memset(t_[64:65, 0:1152].bitcast(F32), 1.0)
            hist.append(t_)
        c_sb = pstate.tile([128, 256], F32, tag="csb")
        nc.vector.memset(c_sb[:], 0.0)
        h2acc = pstate.tile([64, 512], F32, tag="h2acc")
        nc.vector.memset(h2acc[:], 0.0)

        # ---- PSUM gate banks ----
        pg0 = [ppsum.tile([128, 512], F32, tag=f"pg0{i}", name=f"pg0{i}") for i in range(2)]
        pg1 = [ppsum.tile([128, 512], F32, tag=f"pg1{i}", name=f"pg1{i}") for i in range(2)]

        mm = nc.tensor.matmul

        def big_l0(p):
            bank = pg0[p % 2]
            t0 = 2 * p
            ch = xsb[(t0 // 16) % 3]
            rhs = ch[:, (t0 % 16) * 128:(t0 % 16) * 128 + 256]
            rr = bank[:].rearrange("q (s c) -> q s c", c=256)
            mm(rr[:, :, 0:128], w["wxif0"][:], rhs, start=True, stop=False,
               skip_group_check=True)
            mm(rr[:, :, 128:256], w["wxog0"][:], rhs, start=True, stop=False,
               skip_group_check=True)

        def big_l1(q):
            bank = pg1[q % 2]
            t0 = 2 * q
            hb = hist[_blk(t0)]
            a = t0 % 4
            rhs = hb[0:65, 128:1152].rearrange("q (s c) -> q s c", c=256)[:, a:a + 2, 0:128]
            rr = bank[:].rearrange("q (s c) -> q s c", c=256)
            mm(rr[:, :, 0:128], w["w0if1"][:], rhs, start=True, stop=False,
               skip_group_check=True)
            mm(rr[:, :, 128:256], w["w0og1"][:], rhs, start=True, stop=False,
               skip_group_check=True)

        def hh(layer, t):
            # recurrent matmuls for step t of `layer` (skipped at t=0)
            bank = (pg0 if layer == 0 else pg1)[(t // 2) % 2]
            out = bank[:, 256 * (t % 2):256 * (t % 2) + 256]
            hb = hist[_blk(t - 1)]
            base = _h0col(t - 1) if layer == 0 else _h1col(t - 1)
            rhs_if = hb[0:64, base:base + 256]          # [h | 0]
            rhs_og = hb[0:64, base - 128:base + 128]    # [0 | h]
            wif = w["whif0" if layer == 0 else "whif1"]
            wog = w["whog0" if layer == 0 else "whog1"]
            last = (t % 2 == 1)
            mm(out, wif[:], rhs_if, start=False, stop=False, skip_group_check=True)
            mm(out, wog[:], rhs_og, start=False, stop=last, skip_group_check=True)

        for k in range(t_steps + LAG):
            t = k
            tau = k - LAG
            l0 = t < t_steps
            l1 = 0 <= tau < t_steps

            # ---- input streaming ----
            if k % 16 == 0 and k // 16 + 2 < NCH:
                j = k // 16 + 2
                nc.sync.dma_start(xsb[j % 3][:], d_xs.ap()[:, j * 2048:(j + 1) * 2048])
            if k % 16 == 4 and k // 16 + 2 < NCH:
                j = k // 16 + 2
                nc.sync.dma_start(dcb[j % 3][:], d_dbc.ap()[:, j * 2048:(j + 1) * 2048])

            # ---- big (off-chain) projection matmuls, once per pair ----
            if k % 2 == 0:
                if l0:
                    big_l0(t // 2)
                q = k // 2 - LAG // 2
                if 0 <= q < t_steps // 2:
                    big_l1(q)

            # ---- recurrent matmuls ----
            if l0 and t >= 1:
                hh(0, t)
            if l1 and tau >= 1:
                hh(1, tau)

            # ---- activations: one sigmoid per gate block ----
            g_sb = pwork.tile([128, 512], F32, tag="gsb")
            if l0:
                bank = pg0[(t // 2) % 2]
                s = 256 * (t % 2)
                nc.scalar.activation(g_sb[:, 0:128], bank[:, s:s + 128], AF.Sigmoid)
                nc.scalar.activation(g_sb[:, 128:256], bank[:, s + 128:s + 256], AF.Sigmoid)
            if l1:
                bank = pg1[(tau // 2) % 2]
                s = 256 * (tau % 2)
                nc.scalar.activation(g_sb[:, 256:384], bank[:, s:s + 128], AF.Sigmoid)
                nc.scalar.activation(g_sb[:, 384:512], bank[:, s + 128:s + 256], AF.Sigmoid)

            # ---- cell update + hidden write, per layer ----
            tc_sb = pwork.tile([128, 256], F32, tag="tcsb")
            for ell, act, st in ((0, l0, t), (1, l1, tau)):
                if not act:
                    continue
                o0 = 256 * ell
                fco = pwork.tile([64, 128], F32, tag=f"fc{ell}")
                nc.vector.tensor_tensor(fco[:], g_sb[64:128, o0:o0 + 128],
                                        c_sb[64:128, 128 * ell:128 * ell + 128], OP.mult)
                ig2 = pwork.tile([64, 128], F32, tag=f"ig{ell}")
                nc.vector.scalar_tensor_tensor(ig2[:], g_sb[0:64, o0 + 128:o0 + 256],
                                               0.5, g_sb[0:64, o0:o0 + 128],
                                               OP.subtract, OP.mult)
                nc.vector.scalar_tensor_tensor(c_sb[64:128, 128 * ell:128 * ell + 128],
                                               ig2[:], 2.0, fco[:], OP.mult, OP.add)
                nc.scalar.activation(tc_sb[64:128, 128 * ell:128 * ell + 128],
                                     c_sb[64:128, 128 * ell:128 * ell + 128], AF.Tanh)
                hb = hist[_blk(st)]
                dst = _h0col(st) if ell == 0 else _h1col(st)
                nc.vector.tensor_tensor(hb[0:64, dst:dst + 128],
                                        g_sb[64:128, o0 + 128:o0 + 256],
                                        tc_sb[64:128, 128 * ell:128 * ell + 128], OP.mult)

            # ---- capture: h2acc += d_blk * h1_blk, per 4-step block ----
            if k % 4 == 1 and k >= 5:
                c = (k - 5) // 4
                t0 = 4 * c
                ch = dcb[(t0 // 16) % 3]
                dsl = ch[:, (t0 % 16) * 128:(t0 % 16) * 128 + 512]
                hb = hist[_blk(t0)]
                h1s = hb[0:64, 1280:2304].rearrange("q (s c) -> q s c", c=256)[:, :, 0:128]
                mblk = pwork.tile([64, 512], F32, tag="mblk")
                nc.vector.tensor_tensor(mblk[:], dsl, h1s, OP.mult)
                nc.vector.tensor_tensor(h2acc[:], h2acc[:], mblk[:], OP.add)

        # ---------- FC + sigmoid head ----------
        hfold = pwork.tile([64, 256], F32, tag="hfold")
        nc.vector.tensor_tensor(hfold[:], h2acc[:, 0:256], h2acc[:, 256:512], OP.add)
        h2 = pwork.tile([64, 128], F32R, tag="h2")
        nc.vector.tensor_tensor(h2[:], hfold[:, 0:128], hfold[:, 128:256], OP.add)
        pfc = ppsum.tile([1, 128], F32, tag="pfc")
        mm(pfc[:], fct[:], h2[:], start=True, stop=True)
        osb = pwork.tile([1, 128], F32, tag="osb")
        nc.scalar.activation(osb[:], pfc[:], AF.Sigmoid, bias=fcb[:, 0:1])
        nc.sync.dma_start(d_out.ap()[:], osb[:])

    nc.compile()
    return nc


def _get_program(t_steps: int):
    if t_steps not in _BUILT:
        _BUILT[t_steps] = _build_program(t_steps)
    return _BUILT[t_steps]


def _prep_core_inputs(x, dmask, weights, t_steps):
    """Host-side layout prep for one core's shard. x: [BL, T, I], dmask: [BL, T]."""
    TB = t_steps * BL
    xs = np.empty((17, TB), np.float32)
    xs[0:16] = np.asarray(x, np.float32).transpose(2, 1, 0).reshape(16, TB)
    xs[16] = 1.0
    dbc = np.ascontiguousarray(
        np.broadcast_to(dmask.T.reshape(1, TB), (64, TB)).astype(np.float32))
    return dict(xs=xs, dbc=dbc, **weights)


def _host_weights(w_ih0, w_hh0, b_ih0, b_hh0,
                  w_ih1, w_hh1, b_ih1, b_hh1, fc_w, fc_b):
    b0 = np.asarray(b_ih0, np.float32) + np.asarray(b_hh0, np.float32)
    b1 = np.asarray(b_ih1, np.float32) + np.asarray(b_hh1, np.float32)
    wih0, whh0 = np.asarray(w_ih0, np.float32), np.asarray(w_hh0, np.float32)
    wih1, whh1 = np.asarray(w_ih1, np.float32), np.asarray(w_hh1, np.float32)

    def og_w(wm):  # [4H, K] -> [2g; o] stacked [128, K]
        return np.concatenate([2.0 * wm[2 * H:3 * H], wm[3 * H:4 * H]], axis=0)

    def og_b(bv):
        return np.concatenate([2.0 * bv[2 * H:3 * H], bv[3 * H:4 * H]])

    def rider(wm, bv, k):  # lhsT [k, 128] with bias rider in last row
        out = np.zeros((k, 128), np.float32)
        out[0:k - 1] = wm.T
        out[k - 1] = bv
        return out

    weights = dict(
        wxif0=rider(wih0[0:2 * H], b0[0:2 * H], 17),
        wxog0=rider(og_w(wih0), og_b(b0), 17),
        whif0=np.ascontiguousarray(whh0[0:2 * H].T),
        whog0=np.ascontiguousarray(og_w(whh0).T),
        w0if1=rider(wih1[0:2 * H], b1[0:2 * H], 65),
        w0og1=rider(og_w(wih1), og_b(b1), 65),
        whif1=np.ascontiguousarray(whh1[0:2 * H].T),
        whog1=np.ascontiguousarray(og_w(whh1).T),
        fct=np.ascontiguousarray(np.asarray(fc_w, np.float32).reshape(1, H).T),
        fcb=np.asarray(fc_b, np.float32).reshape(1, 1),
    )
    return weights


def _run(x, mask, w_ih0, w_hh0, b_ih0, b_hh0,
         w_ih1, w_hh1, b_ih1, b_hh1, fc_w, fc_b, trace=False):
    t_steps = x.shape[1]
    x = np.asarray(x, np.float32)
    mask = np.asarray(mask)

    # d[b, t] = mask[b, t] - mask[b, t+1]  (one-hot at t = len_b - 1)
    m = mask.astype(np.float32)
    d = m - np.concatenate([m[:, 1:], np.zeros((m.shape[0], 1), np.float32)], axis=1)

    weights = _host_weights(w_ih0, w_hh0, b_ih0, b_hh0,
                            w_ih1, w_hh1, b_ih1, b_hh1, fc_w, fc_b)

    nc = _get_program(t_steps)
    in_maps = []
    for c in range(NCORES):
        sl = slice(c * BL, (c + 1) * BL)
        in_maps.append(_prep_core_inputs(x[sl], d[sl], weights, t_steps))

    res = bass_utils.run_bass_kernel_spmd(nc, in_maps, core_ids=list(range(NCORES)),
                                          trace=trace)
    out = np.concatenate([res.results[c]["out"].reshape(BL) for c in range(NCORES)])
    return out.astype(np.float32), res


def kernel(**inputs):
    return _run(**inputs)[0]


def kernel_traced(**inputs):
    return _run(**inputs, trace=True)
